# revision 19
# baseline (speedup 1.0000x reference)
"""MGE velocity kernel for 8 Trainium2 NeuronCores.

Reference math per point: v = R_sc * sqrt(vc2_mge(r2) + vc2_bh(r2)) with
r2 = x^2+y^2+z^2 (unscaled), vc2_bh = bh_c * r2^-1.5, and vc2_mge a
positive sum of decaying exponentials in r2 (MGE quadrature).

Host-side analysis (exact, from the small parameter vectors + the data's
r2 range) computes ratio = vc2_mge/vc2_bh over the data's r2 interval.
For the staged inputs m_bh=8 makes the black-hole term dominate:
max ratio ~ 6.1e-5, so dropping the MGE sum and folding a constant
correction sqrt(1+mean_ratio) into the prefactor gives max rel err
~1.6e-5 -- below even the baseline kernel's 1.7e-5.

Fast path (BH-only), per core (131072 points = [128, 1024] fp32):
    v = K * r2^-0.25      (K = sqrt(G*10^m_bh), corrected)
  - inputs converted host-side to fp16 and packed chunk-interleaved
    [x_c|y_c|z_c]*NCH so each chunk is one contiguous DMA
  - DVE (fp16 2x): y^2, z^2, two adds;  ACT: x^2, Ln, Exp (one table set)
  - out fp16 [128,1024], host upcasts to fp32
  Rel err budget: fp16 input quantization ~5e-4 -> v err ~9e-4 max
  (verified vs reference in fp64/numpy), harness gate is 2e-2.

General path (taken when host analysis finds the MGE sum matters at
>1e-3): NNLS re-fit of the exponential mixture on a log-spaced b-grid
(M' terms, typically <=16 vs the reference's 2048), evaluated as M'
extra ACT Exp passes accumulated on DVE, plus the exact BH term.
"""

import numpy as np
from numpy.polynomial.legendre import leggauss

N_CORES = 8
H = W = 1024
N = H * W
P = 128
FN = N // N_CORES // P    # 1024 columns per core
NCH = 4                   # input chunks (DMA/compute pipeline)
CW = FN // NCH
G_CONST = 0.004301

_CACHE = {}


def _register_consts(nc, mybir, vals):
    """Make float values usable as activation bias= immediates."""
    fp32 = mybir.dt.float32
    for i, v in enumerate(vals):
        v = float(v)
        if (fp32, v) in nc.const_aps.aps:
            continue
        t = nc.alloc_sbuf_tensor(f"kconst_{i}", [128, 1], fp32)
        nc.gpsimd.memset(t.ap(), v)
        nc.const_aps.aps[(fp32, v)] = t.ap()


BH_SIZES = (256, 512, 256)   # out chunks; each must divide FN (kv ncn)
BH_CSIZES = (256, 512, 256)   # compute chunks (bounds must cover out bounds)


def _build_bh(lnK, sizes=BH_SIZES, csizes=BH_CSIZES):
    """BH-only kernel: out = K * r2^-0.25 with K = exp(lnK).

    Raw bass (no TileContext) with manual semaphores:
      - v = recip(sqrt(sqrt(r2) / K^2)): squares/adds in fp16 (DVE 2x),
        two ACT Sqrt passes (one activation table), then
        reciprocal_approx_fast (single DVE op, ~51 ULP, fp32->fp32)
      - compute is pipelined over csizes column chunks; outputs go out in
        (fewer) sizes chunks via SWDGE kv_writeback descriptors prepared
        up-front on GPSIMD (data-independent) and fired by trigger_dma,
        removing the per-DMA HWDGE issue chain from the kernel tail
      - a standard DVE copy after the custom reciprocal carries the
        v_sem increment (custom-op then_inc raced the out DMA on HW)
    """
    key = ("bh", round(float(lnK), 7), tuple(sizes), tuple(csizes))
    if key in _CACHE:
        return _CACHE[key]
    import concourse.mybir as mybir
    from concourse import bacc

    fp16 = mybir.dt.float16
    fp32 = mybir.dt.float32
    i32 = mybir.dt.int32
    AF = mybir.ActivationFunctionType
    OP = mybir.AluOpType

    K2inv = float(np.exp(-2.0 * float(lnK)))
    nch = len(csizes)
    noc = len(sizes)
    offs = np.concatenate([[0], np.cumsum(csizes)]).astype(int)
    ooffs = np.concatenate([[0], np.cumsum(sizes)]).astype(int)
    assert offs[-1] == FN and ooffs[-1] == FN
    # out chunk i fires once compute chunks covering it are done
    vneed = [int(np.searchsorted(offs, ooffs[i + 1])) for i in range(noc)]
    assert all(offs[vneed[i]] == ooffs[i + 1] for i in range(noc)), \
        "out chunk bounds must align with compute chunk bounds"
    nc = bacc.Bacc("TRN2")
    xyz = nc.dram_tensor("xyz", [P, 3 * FN], fp16, kind="ExternalInput")
    out = nc.dram_tensor("out", [1, P, 1, FN], fp32, kind="ExternalOutput")

    xyz_t = nc.alloc_sbuf_tensor("xyz_t", [P, 3 * FN], fp16)
    sx = nc.alloc_sbuf_tensor("sx_t", [P, FN], fp16)
    sy = nc.alloc_sbuf_tensor("sy_t", [P, FN], fp16)
    r2 = nc.alloc_sbuf_tensor("r2_t", [P, FN], fp16)
    s1 = nc.alloc_sbuf_tensor("s1_t", [P, FN], fp16)
    s2 = nc.alloc_sbuf_tensor("s2_t", [P, FN], fp32)
    vr = nc.alloc_sbuf_tensor("vr_t", [P, FN], fp32)
    v = nc.alloc_sbuf_tensor("v_t", [P, 1, 1, FN], fp32)
    idx = nc.alloc_sbuf_tensor("idx_t", [P, noc], i32)

    in_sem = nc.alloc_semaphore("in_sem")
    r2_sem = nc.alloc_semaphore("r2_sem")
    s2_sem = nc.alloc_semaphore("s2_sem")
    v_sem = nc.alloc_semaphore("v_sem")
    prep_sem = nc.alloc_semaphore("prep_sem")
    dma_sem = nc.alloc_semaphore("dma_sem")

    # SP queue: chunked input DMAs (complete in issue order)
    for c in range(nch):
        o0, o1 = 3 * offs[c], 3 * offs[c + 1]
        nc.sync.dma_start(xyz_t[:, o0:o1], xyz[:, o0:o1]).then_inc(in_sem, 16)

    # GPSIMD: idx constants and out-descriptor preps (data-independent,
    # run while inputs stream in), then the per-out-chunk triggers
    for i in range(noc):
        nc.gpsimd.memset(idx[:, i : i + 1], int(ooffs[i]))
    for i in range(noc):
        nc.gpsimd.kv_writeback(
            out[:, :, :, :],
            v[:, :, :, ooffs[i] : ooffs[i + 1]],
            idx[:, i : i + 1],
            prepare_only=True,
            sem=dma_sem,
        ).then_inc(prep_sem, 16)
    nc.gpsimd.wait_ge(prep_sem, 16 * noc)
    for i in range(noc):
        nc.gpsimd.wait_ge(v_sem, 16 * vneed[i])
        nc.gpsimd.trigger_dma(count=1)

    # DVE: squares/adds per chunk; previous chunk's reciprocal interleaved
    def recip(c):
        sl = slice(offs[c], offs[c + 1])
        nc.vector.wait_ge(s2_sem, 16 * (c + 1))
        # the custom-op's SBUF write drain is not ordered with its (or a
        # successor's) then_inc as observed by SWDGE readers (first-exec
        # NaNs on HW), so never let the out DMA read the custom op's
        # output: write to scratch, then a standard full copy feeds v
        nc.vector.reciprocal_approx_fast(out=vr[:, sl], in_=s2[:, sl])
        nc.vector.tensor_copy(v[:, 0, 0, sl], vr[:, sl]).then_inc(v_sem, 16)

    for c in range(nch):
        o0, w = 3 * offs[c], int(csizes[c])
        sl = slice(offs[c], offs[c + 1])
        x_ = xyz_t[:, o0 : o0 + w]
        y_ = xyz_t[:, o0 + w : o0 + 2 * w]
        z_ = xyz_t[:, o0 + 2 * w : o0 + 3 * w]
        nc.vector.wait_ge(in_sem, 16 * (c + 1))
        nc.vector.tensor_tensor(sx[:, sl], x_, x_, OP.mult)
        nc.vector.tensor_tensor(sy[:, sl], y_, y_, OP.mult)
        nc.vector.tensor_tensor(r2[:, sl], z_, z_, OP.mult)
        nc.vector.tensor_tensor(sy[:, sl], sy[:, sl], sx[:, sl], OP.add)
        nc.vector.tensor_tensor(r2[:, sl], r2[:, sl], sy[:, sl], OP.add).then_inc(
            r2_sem, 16
        )
        if c > 0:
            recip(c - 1)
    recip(nch - 1)

    # ACT: the two Sqrt passes per chunk (single activation table)
    for c in range(nch):
        sl = slice(offs[c], offs[c + 1])
        nc.scalar.wait_ge(r2_sem, 16 * (c + 1))
        nc.scalar.activation(s1[:, sl], r2[:, sl], AF.Sqrt)
        nc.scalar.activation(s2[:, sl], s1[:, sl], AF.Sqrt, scale=K2inv).then_inc(
            s2_sem, 16
        )

    # hold kernel completion until every out DMA has landed, then clear
    # semaphore/DMA state so repeat executions of the NEFF start clean
    nc.sync.wait_ge(dma_sem, 16 * noc)
    nc.reset()
    nc.compile()
    _CACHE[key] = nc
    return nc


def _build_mge(bs, lncs, ln_bhc, ln_vsc, n_chunks=NCH):
    """General kernel: vc2 = sum_m exp(-b_m*r2 + lnc_m) + exp(-1.5*ln r2
    + ln_bhc); out = exp(0.5*ln(vc2*r2) + ln_vsc)."""
    key = ("mge", tuple(np.round(bs, 10)), tuple(np.round(lncs, 7)),
           round(float(ln_bhc), 7), round(float(ln_vsc), 7), n_chunks)
    if key in _CACHE:
        return _CACHE[key]
    import concourse.mybir as mybir
    from concourse import bacc
    from concourse.tile import TileContext

    fp32 = mybir.dt.float32
    fp16 = mybir.dt.float16
    AF = mybir.ActivationFunctionType
    OP = mybir.AluOpType

    cw = FN // n_chunks
    nc = bacc.Bacc("TRN2")
    _register_consts(
        nc, mybir,
        [float(ln_bhc), float(ln_vsc)] + [float(v) for v in lncs],
    )
    xyz = nc.dram_tensor("xyz", [P, 3 * FN], fp16, kind="ExternalInput")
    out = nc.dram_tensor("out", [P, FN], fp16, kind="ExternalOutput")
    with TileContext(nc) as tc:
        with tc.tile_pool(name="s", bufs=1) as s:
            xyz_t = s.tile([P, 3 * FN], fp16)
            sx = s.tile([P, FN], fp16)
            sy = s.tile([P, FN], fp16)
            r2 = s.tile([P, FN], fp16)
            lr = s.tile([P, FN], fp32)
            acc = s.tile([P, FN], fp32)
            em = s.tile([P, FN], fp32)
            tv = s.tile([P, FN], fp32)
            v = s.tile([P, FN], fp16)
            for c in range(n_chunks):
                nc.sync.dma_start(
                    xyz_t[:, 3 * cw * c : 3 * cw * (c + 1)],
                    xyz[:, 3 * cw * c : 3 * cw * (c + 1)],
                )
            for c in range(n_chunks):
                x_ = xyz_t[:, 3 * cw * c : 3 * cw * c + cw]
                y_ = xyz_t[:, 3 * cw * c + cw : 3 * cw * c + 2 * cw]
                z_ = xyz_t[:, 3 * cw * c + 2 * cw : 3 * cw * (c + 1)]
                sl = slice(cw * c, cw * (c + 1))
                nc.scalar.activation(sx[:, sl], x_, AF.Square)
                nc.vector.tensor_tensor(sy[:, sl], y_, y_, OP.mult)
                nc.vector.tensor_tensor(r2[:, sl], z_, z_, OP.mult)
                nc.vector.tensor_tensor(sy[:, sl], sy[:, sl], sx[:, sl], OP.add)
                nc.vector.tensor_tensor(r2[:, sl], r2[:, sl], sy[:, sl], OP.add)
                nc.scalar.activation(lr[:, sl], r2[:, sl], AF.Ln)
                # vc2_bh = exp(-1.5*ln r2 + ln_bhc)
                nc.scalar.activation(
                    acc[:, sl], lr[:, sl], AF.Exp, bias=float(ln_bhc), scale=-1.5
                )
                # accumulate the refit exponential terms
                for b_m, lnc_m in zip(bs, lncs):
                    nc.scalar.activation(
                        em[:, sl], r2[:, sl], AF.Exp,
                        bias=float(lnc_m), scale=float(-b_m),
                    )
                    nc.vector.tensor_tensor(
                        acc[:, sl], acc[:, sl], em[:, sl], OP.add
                    )
                # v = exp(0.5*ln(vc2 * r2) + ln_vsc)
                nc.vector.tensor_tensor(tv[:, sl], acc[:, sl], r2[:, sl], OP.mult)
                nc.scalar.activation(lr[:, sl], tv[:, sl], AF.Ln)
                nc.scalar.activation(
                    v[:, sl], lr[:, sl], AF.Exp, bias=float(ln_vsc), scale=0.5
                )
                nc.sync.dma_start(out[:, sl], v[:, sl])
    nc.compile()
    _CACHE[key] = nc
    return nc


def _exact_terms(surf, sigma, qobs, M_to_L, inc, quad=64):
    """Converged (b, c) exponential decomposition of vc2_mge in unscaled
    r2 units, mirroring reference.py's math in fp64."""
    surf = surf.astype(np.float64)
    sigma = sigma.astype(np.float64)
    qobs = qobs.astype(np.float64)
    cos_i, sin_i = np.cos(inc), np.sin(inc)
    q_intr = np.sqrt(qobs**2 - cos_i**2) / sin_i
    md = surf * M_to_L * qobs / (q_intr * sigma * np.sqrt(2.0 * np.pi))
    scale = np.quantile(sigma, 0.5)
    sig_sc = sigma / scale
    mds = np.quantile(sig_sc, 0.5)
    mxs = sig_sc.max()
    t_lo = np.arcsinh(np.log(1e-7 * mds) * 2.0 / np.pi)
    t_hi = np.arcsinh(np.log(1000.0 * mxs) * 2.0 / np.pi)
    xl, wl = leggauss(quad)
    t = 0.5 * (t_hi - t_lo) * xl + 0.5 * (t_hi + t_lo)
    w = 0.5 * (t_hi - t_lo) * wl
    u = np.exp(np.pi / 2.0 * np.sinh(t))
    du = np.pi / 2.0 * np.cosh(t) * u
    coef = q_intr * md
    inv_s2 = 1.0 / sig_sc**2
    a_j = 0.5 / (1.0 + u)
    b = (a_j[:, None] * inv_s2[None, :]).ravel() / scale**2
    c = ((coef[None, :] / ((1.0 + u[:, None]) ** 2
                           * np.sqrt(q_intr[None, :] ** 2 + u[:, None])))
         * (du * w)[:, None]).ravel()
    c = c * 2.0 * np.pi * G_CONST * scale**2      # direct vc2_mge scale
    return b, c, scale


def _f_of(b, c, r2v):
    return (c[None, :] * np.exp(-np.minimum(b[None, :] * r2v[:, None], 700.0))).sum(1)


def _refit(b, c, samp, wgt, max_terms=24, tol=2e-4):
    """NNLS re-fit of sum_m c_m exp(-b_m r2) on a log-spaced b-grid with
    relative-to-total weighting. Returns the smallest grid whose fit
    meets tol (relative to total vc2)."""
    from scipy.optimize import nnls
    f = _f_of(b, c, samp)
    target = f * wgt
    for nb in (6, 8, 12, 16, 24, 32, 48):
        bgrid = np.geomspace(max(b.min(), 1e-8), b.max() * 1.5, nb)
        A = np.exp(-np.minimum(bgrid[None, :] * samp[:, None], 700.0)) * wgt[:, None]
        coefs, _ = nnls(A, target)
        nz = coefs > 0
        fit = _f_of(bgrid[nz], coefs[nz], samp)
        if (np.abs(fit - f) * wgt).max() < tol and nz.sum() <= max_terms:
            return bgrid[nz], coefs[nz]
    return bgrid[nz], coefs[nz]     # best effort


def kernel(x, y, z, surf, sigma, qobs, M_to_L, inc, m_bh, quad_points):
    from concourse.bass_utils import run_bass_kernel_spmd

    x = np.asarray(x, dtype=np.float32)
    y = np.asarray(y, dtype=np.float32)
    z = np.asarray(z, dtype=np.float32)
    b, c, scale = _exact_terms(
        np.asarray(surf), np.asarray(sigma), np.asarray(qobs),
        float(M_to_L), float(inc),
    )
    bh_c = G_CONST * 10.0 ** float(m_bh) * scale**2   # vc2_bh = bh_c * r2^-1.5

    # data r2 range (host O(N) pass; informs the approximation choice only)
    r2f = (x.astype(np.float64) ** 2 + y.astype(np.float64) ** 2
           + z.astype(np.float64) ** 2)
    r2min = max(float(r2f.min()), 1e-12)
    r2max = float(r2f.max())
    samp = np.geomspace(r2min, r2max, 512)
    fs = _f_of(b, c, samp)
    bhs = bh_c * samp**-1.5
    ratio = fs / bhs
    rmin, rmax = float(ratio.min()), float(ratio.max())

    if 0.25 * (rmax - rmin) < 1e-3:
        # BH term dominates: v = K * r2^-0.25 with constant mge correction
        lnK = 0.5 * (np.log(G_CONST) + float(m_bh) * np.log(10.0)) \
            + 0.5 * np.log1p(0.5 * (rmin + rmax))
        nc = _build_bh(lnK)
        sizes = BH_CSIZES
    else:
        wgt = 1.0 / (fs + bhs)
        bs, cs = _refit(b, c, samp, wgt)
        ln_bhc = np.log(bh_c)
        ln_vsc = -np.log(scale)
        nc = _build_mge(bs, np.log(cs), ln_bhc, ln_vsc)
        sizes = (CW,) * NCH

    # pack fp16 chunk-interleaved [x_c|y_c|z_c] per core
    offs = np.concatenate([[0], np.cumsum(sizes)]).astype(int)
    xf = x.ravel().reshape(N_CORES, P, FN)
    yf = y.ravel().reshape(N_CORES, P, FN)
    zf = z.ravel().reshape(N_CORES, P, FN)
    xyzc = np.empty((N_CORES, P, 3 * FN), np.float16)
    for c in range(len(sizes)):
        a, b2 = offs[c], offs[c + 1]
        w = b2 - a
        xyzc[:, :, 3 * a : 3 * a + w] = xf[:, :, a:b2]
        xyzc[:, :, 3 * a + w : 3 * a + 2 * w] = yf[:, :, a:b2]
        xyzc[:, :, 3 * a + 2 * w : 3 * b2] = zf[:, :, a:b2]

    in_maps = [{"xyz": xyzc[i]} for i in range(N_CORES)]
    res = run_bass_kernel_spmd(nc, in_maps, core_ids=list(range(N_CORES)))
    outs = [res.results[i]["out"].astype(np.float32).reshape(-1)
            for i in range(N_CORES)]
    _CACHE["last_nc"] = nc
    return np.concatenate(outs).reshape(H, W)
